# revision 1
# baseline (speedup 1.0000x reference)
"""BCMSELoss (periodic-angle MSE + constant penalty) on 8 TRN2 NeuronCores.

Pure data parallel: the batch dim (8,388,608 rows of 3 floats) is split into
8 shards of 1,048,576 rows; each core streams its 2 x 12 MiB shard through
SBUF in tiles and reduces three per-partition partial sums:

  - angle-cols squared wrap error:  sum((u - rint(u))^2),  u = o - t
  - penalty:                        sum(|floor(o)|)        (angle cols)
  - col0 squared error:             sum((o - t)^2)

The reference's wrap-shift (move target by +-1 when |mod(o,1) - t| > 0.5) is
algebraically u - rint(u) applied to the raw difference u = o - t; rint is
computed exactly in fp32 with the magic-number trick (x + 1.5*2^23) - 1.5*2^23
(round-half-even == jnp semantics at the measure-zero tie points after
squaring), and floor(x) = rint(x - 0.5), exact except x exactly integral
(probability ~2^-24 per element; perturbs the penalty by at most 1/B each).

Engine schedule per tile (all elementwise work on DVE, reductions on ACT):
  DVE: u = o_ang - t_ang          (tensor_tensor, strided col view)
       r = (u + M) - M            (dual-op tensor_scalar, 2x mode)
       -d2 = r - u                (tensor_tensor)
       s2 = (o_ang - 0.5) + M     (dual-op tensor_scalar -> M + floor(o))
       u0 = o_0 - t_0             (tensor_tensor, strided col view)
  ACT: Square(-d2)  + accum       -> angle sq partial
       Abs(s2 - M)  + accum       -> penalty partial
       Square(u0)   + accum       -> col0 sq partial
GPSIMD is intentionally unused (its tensor ops measured ~5x slower than the
cost model on hardware); DMA loads go through the SP HWDGE ring.

Per-core output is a [128, 3*NT] fp32 accumulator; the host sums in float64
and combines: loss = sq_total / (B*3) + penalty_total / B.
"""
import sys

sys.path.insert(0, "/opt/trn_rl_repo")

import numpy as np

B = 8388608
C = 3
NCORES = 8
P = 128
BP = B // NCORES                   # rows per core
FLAT = BP * C                      # 3,145,728 f32 per tensor per core
PER_PART = FLAT // P               # 24,576 elements per partition
MAGIC = 12582912.0                 # 1.5 * 2**23

SIZES = [384] + [1536] * 15 + [768, 384]
assert sum(SIZES) == PER_PART

_CACHE = {}


def _build_program():
    import concourse.bacc as bacc
    import concourse.tile as tile
    from concourse import mybir

    nt = len(SIZES)
    nc = bacc.Bacc("TRN2", target_bir_lowering=False, debug=False)

    o_d = nc.dram_tensor("outputs", [BP, C], mybir.dt.float32, kind="ExternalInput").ap()
    t_d = nc.dram_tensor("targets", [BP, C], mybir.dt.float32, kind="ExternalInput").ap()
    acc_d = nc.dram_tensor("acc", [P, 3 * nt], mybir.dt.float32, kind="ExternalOutput").ap()

    o2 = o_d.flatten().rearrange("(p m) -> p m", p=P)
    t2 = t_d.flatten().rearrange("(p m) -> p m", p=P)

    f32 = mybir.dt.float32
    AO = mybir.AluOpType
    AF = mybir.ActivationFunctionType

    with tile.TileContext(nc) as tc:
        with (
            tc.tile_pool(name="io", bufs=6) as io_pool,
            tc.tile_pool(name="work", bufs=3) as w_pool,
            tc.tile_pool(name="fixed", bufs=1) as f_pool,
        ):
            neg_magic = f_pool.tile([P, 1], f32)
            nc.vector.memset(neg_magic[:], -MAGIC)
            acc = f_pool.tile([P, 3 * nt], f32)

            off = 0
            for k, s in enumerate(SIZES):
                sa, s0 = s // 3 * 2, s // 3
                o = io_pool.tile([P, s], f32, tag="o")
                t = io_pool.tile([P, s], f32, tag="t")
                nc.sync.dma_start(o[:], o2[:, off:off + s])
                nc.sync.dma_start(t[:], t2[:, off:off + s])
                off += s

                orr = o[:].rearrange("p (n c) -> p n c", c=3)
                trr = t[:].rearrange("p (n c) -> p n c", c=3)
                oa, ta = orr[:, :, 1:3], trr[:, :, 1:3]
                o0, t0 = orr[:, :, 0], trr[:, :, 0]

                # angle squared wrap-error
                u = w_pool.tile([P, sa], f32, tag="u")
                nc.vector.tensor_tensor(
                    u[:].rearrange("p (n c) -> p n c", c=2), oa, ta, AO.subtract
                )
                r = w_pool.tile([P, sa], f32, tag="r")
                nc.vector.tensor_scalar(r[:], u[:], MAGIC, MAGIC, AO.add, AO.subtract)
                negd2 = w_pool.tile([P, sa], f32, tag="negd2")
                nc.vector.tensor_tensor(negd2[:], r[:], u[:], AO.subtract)
                nc.scalar.activation(
                    negd2[:], negd2[:], AF.Square, accum_out=acc[:, 3 * k: 3 * k + 1]
                )

                # penalty: |floor(o_angle)| via M + floor(o) then Abs(x - M)
                s2 = w_pool.tile([P, sa], f32, tag="s2")
                nc.vector.tensor_scalar(
                    s2[:].rearrange("p (n c) -> p n c", c=2),
                    oa, 0.5, MAGIC, AO.subtract, AO.add,
                )
                nc.scalar.activation(
                    s2[:], s2[:], AF.Abs, bias=neg_magic[:], scale=1.0,
                    accum_out=acc[:, 3 * k + 1: 3 * k + 2],
                )

                # col0 squared error
                u0 = w_pool.tile([P, s0], f32, tag="u0")
                nc.vector.tensor_tensor(u0[:], o0, t0, AO.subtract)
                nc.scalar.activation(
                    u0[:], u0[:], AF.Square, accum_out=acc[:, 3 * k + 2: 3 * k + 3]
                )

            nc.sync.dma_start(acc_d, acc[:])

    nc.compile()
    return nc


def _get_program():
    if "nc" not in _CACHE:
        _CACHE["nc"] = _build_program()
    return _CACHE["nc"]


def kernel(outputs: np.ndarray, targets: np.ndarray) -> np.ndarray:
    from concourse.bass_utils import run_bass_kernel_spmd

    assert outputs.shape == (B, C) and targets.shape == (B, C)
    nc = _get_program()

    o_sh = np.ascontiguousarray(np.asarray(outputs, dtype=np.float32).reshape(NCORES, BP, C))
    t_sh = np.ascontiguousarray(np.asarray(targets, dtype=np.float32).reshape(NCORES, BP, C))
    in_maps = [{"outputs": o_sh[i], "targets": t_sh[i]} for i in range(NCORES)]

    res = run_bass_kernel_spmd(nc, in_maps, core_ids=list(range(NCORES)))

    nt = len(SIZES)
    sq = 0.0
    pen = 0.0
    for i in range(NCORES):
        a = res.results[i]["acc"].astype(np.float64).reshape(P, nt, 3)
        sq += a[:, :, 0].sum() + a[:, :, 2].sum()
        pen += a[:, :, 1].sum()

    result = sq / (B * C) + pen / B
    return np.float32(result)


if __name__ == "__main__":
    rng = np.random.default_rng(0)
    o = rng.standard_normal((B, C)).astype(np.float32)
    t = rng.random((B, C), dtype=np.float32)
    print(kernel(o, t))



# revision 4
# speedup vs baseline: 1.1539x; 1.1539x over previous
"""BCMSELoss (periodic-angle MSE + constant penalty) on 8 TRN2 NeuronCores.

Pure data parallel: the batch dim (8,388,608 rows of 3 floats) is split into
8 shards of 1,048,576 rows; each core streams its 2 x 12 MiB shard through
SBUF in tiles and reduces three per-partition partial sums:

  - angle-cols squared wrap error:  sum((u - rint(u))^2),  u = o - t
  - penalty:                        sum(|floor(o)|)        (angle cols)
  - col0 squared error:             sum((o - t)^2)

The reference's wrap-shift (move target by +-1 when |mod(o,1) - t| > 0.5) is
algebraically u - rint(u) applied to the raw difference u = o - t; rint is
computed exactly in fp32 with the magic-number trick (x + 1.5*2^23) - 1.5*2^23
(round-half-even == jnp semantics at the measure-zero tie points after
squaring), and floor(x) = rint(x - 0.5), exact except x exactly integral
(probability ~2^-24 per element; perturbs the penalty by at most 1/B each).

Engine schedule per tile (all elementwise work on DVE, reductions on ACT):
  DVE: u = o_ang - t_ang          (tensor_tensor, strided col view)
       r = (u + M) - M            (dual-op tensor_scalar, 2x mode)
       -d2 = r - u                (tensor_tensor)
       s2 = (o_ang - 0.5) + M     (dual-op tensor_scalar -> M + floor(o))
       u0 = o_0 - t_0             (tensor_tensor, strided col view)
  ACT: Square(-d2)  + accum       -> angle sq partial
       Abs(s2 - M)  + accum       -> penalty partial
       Square(u0)   + accum       -> col0 sq partial
GPSIMD is intentionally unused (its tensor ops measured ~5x slower than the
cost model on hardware); DMA loads go through the SP HWDGE ring.

The kernel is DMA-bound: compute hides fully under the HBM->SBUF stream
(~320-340 GB/s per core sustained). 3072-wide tiles (1.5 MiB per DMA)
measured fastest; small lead-in/lead-out tiles shorten ramp and drain.
Queue splits (ACT HWDGE / SWDGE), fp32->bf16 cast-on-DMA, and deeper
buffering were all measured slower or neutral.

Per-core output is a [128, 3*NT] fp32 accumulator; the host sums in float64
and combines: loss = sq_total / (B*3) + penalty_total / B.
"""
import sys

sys.path.insert(0, "/opt/trn_rl_repo")

import numpy as np

B = 8388608
C = 3
NCORES = 8
P = 128
BP = B // NCORES                   # rows per core
FLAT = BP * C                      # 3,145,728 f32 per tensor per core
PER_PART = FLAT // P               # 24,576 elements per partition
MAGIC = 12582912.0                 # 1.5 * 2**23

SIZES = [768] + [3072] * 7 + [1536, 768]
assert sum(SIZES) == PER_PART

_CACHE = {}


def _build_program():
    import concourse.bacc as bacc
    import concourse.tile as tile
    from concourse import mybir

    nt = len(SIZES)
    nc = bacc.Bacc("TRN2", target_bir_lowering=False, debug=False)

    o_d = nc.dram_tensor("outputs", [BP, C], mybir.dt.float32, kind="ExternalInput").ap()
    t_d = nc.dram_tensor("targets", [BP, C], mybir.dt.float32, kind="ExternalInput").ap()
    acc_d = nc.dram_tensor("acc", [P, 3 * nt], mybir.dt.float32, kind="ExternalOutput").ap()

    o2 = o_d.flatten().rearrange("(p m) -> p m", p=P)
    t2 = t_d.flatten().rearrange("(p m) -> p m", p=P)

    f32 = mybir.dt.float32
    AO = mybir.AluOpType
    AF = mybir.ActivationFunctionType

    with tile.TileContext(nc) as tc:
        with (
            tc.tile_pool(name="io", bufs=3) as io_pool,
            tc.tile_pool(name="work", bufs=3) as w_pool,
            tc.tile_pool(name="fixed", bufs=1) as f_pool,
        ):
            neg_magic = f_pool.tile([P, 1], f32)
            nc.vector.memset(neg_magic[:], -MAGIC)
            acc = f_pool.tile([P, 3 * nt], f32)

            off = 0
            for k, s in enumerate(SIZES):
                sa, s0 = s // 3 * 2, s // 3
                o = io_pool.tile([P, s], f32, tag="o")
                t = io_pool.tile([P, s], f32, tag="t")
                nc.sync.dma_start(o[:], o2[:, off:off + s])
                nc.sync.dma_start(t[:], t2[:, off:off + s])
                off += s

                orr = o[:].rearrange("p (n c) -> p n c", c=3)
                trr = t[:].rearrange("p (n c) -> p n c", c=3)
                oa, ta = orr[:, :, 1:3], trr[:, :, 1:3]
                o0, t0 = orr[:, :, 0], trr[:, :, 0]

                # angle squared wrap-error
                u = w_pool.tile([P, sa], f32, tag="u")
                nc.vector.tensor_tensor(
                    u[:].rearrange("p (n c) -> p n c", c=2), oa, ta, AO.subtract
                )
                r = w_pool.tile([P, sa], f32, tag="r")
                nc.vector.tensor_scalar(r[:], u[:], MAGIC, MAGIC, AO.add, AO.subtract)
                negd2 = w_pool.tile([P, sa], f32, tag="negd2")
                nc.vector.tensor_tensor(negd2[:], r[:], u[:], AO.subtract)
                nc.scalar.activation(
                    negd2[:], negd2[:], AF.Square, accum_out=acc[:, 3 * k: 3 * k + 1]
                )

                # penalty: |floor(o_angle)| via M + floor(o) then Abs(x - M)
                s2 = w_pool.tile([P, sa], f32, tag="s2")
                nc.vector.tensor_scalar(
                    s2[:].rearrange("p (n c) -> p n c", c=2),
                    oa, 0.5, MAGIC, AO.subtract, AO.add,
                )
                nc.scalar.activation(
                    s2[:], s2[:], AF.Abs, bias=neg_magic[:], scale=1.0,
                    accum_out=acc[:, 3 * k + 1: 3 * k + 2],
                )

                # col0 squared error
                u0 = w_pool.tile([P, s0], f32, tag="u0")
                nc.vector.tensor_tensor(u0[:], o0, t0, AO.subtract)
                nc.scalar.activation(
                    u0[:], u0[:], AF.Square, accum_out=acc[:, 3 * k + 2: 3 * k + 3]
                )

            nc.sync.dma_start(acc_d, acc[:])

    nc.compile()
    return nc


def _get_program():
    if "nc" not in _CACHE:
        _CACHE["nc"] = _build_program()
    return _CACHE["nc"]


def kernel(outputs: np.ndarray, targets: np.ndarray) -> np.ndarray:
    from concourse.bass_utils import run_bass_kernel_spmd

    assert outputs.shape == (B, C) and targets.shape == (B, C)
    nc = _get_program()

    o_sh = np.ascontiguousarray(np.asarray(outputs, dtype=np.float32).reshape(NCORES, BP, C))
    t_sh = np.ascontiguousarray(np.asarray(targets, dtype=np.float32).reshape(NCORES, BP, C))
    in_maps = [{"outputs": o_sh[i], "targets": t_sh[i]} for i in range(NCORES)]

    res = run_bass_kernel_spmd(nc, in_maps, core_ids=list(range(NCORES)))

    nt = len(SIZES)
    sq = 0.0
    pen = 0.0
    for i in range(NCORES):
        a = res.results[i]["acc"].astype(np.float64).reshape(P, nt, 3)
        sq += a[:, :, 0].sum() + a[:, :, 2].sum()
        pen += a[:, :, 1].sum()

    result = sq / (B * C) + pen / B
    return np.float32(result)


if __name__ == "__main__":
    rng = np.random.default_rng(0)
    o = rng.standard_normal((B, C)).astype(np.float32)
    t = rng.random((B, C), dtype=np.float32)
    print(kernel(o, t))



# revision 9
# speedup vs baseline: 1.1635x; 1.0084x over previous
"""BCMSELoss (periodic-angle MSE + constant penalty) on 8 TRN2 NeuronCores.

Pure data parallel: the batch dim (8,388,608 rows of 3 floats) is split into
8 shards of 1,048,576 rows; each core streams its 2 x 12 MiB shard through
SBUF in tiles and reduces three per-partition partial sums:

  - angle-cols squared wrap error:  sum((u - rint(u))^2),  u = o - t
  - penalty:                        sum(|floor(o)|)        (angle cols)
  - col0 squared error:             sum((o - t)^2)

The reference's wrap-shift (move target by +-1 when |mod(o,1) - t| > 0.5) is
algebraically u - rint(u) applied to the raw difference u = o - t; rint is
computed exactly in fp32 with the magic-number trick (x + 1.5*2^23) - 1.5*2^23
(round-half-even == jnp semantics at the measure-zero tie points after
squaring), and floor(x) = rint(x - 0.5), exact except x exactly integral
(probability ~2^-24 per element; perturbs the penalty by at most 1/B each).

Engine schedule per tile (all elementwise work on DVE, reductions on ACT):
  DVE: u = o_ang - t_ang          (tensor_tensor, strided col view)
       r = (u + M) - M            (dual-op tensor_scalar, 2x mode)
       -d2 = r - u                (tensor_tensor)
       s2 = (o_ang - 0.5) + M     (dual-op tensor_scalar -> M + floor(o))
       u0 = o_0 - t_0             (tensor_tensor, strided col view)
  ACT: Square(-d2)  + accum       -> angle sq partial
       Abs(s2 - M)  + accum       -> penalty partial
       Square(u0)   + accum       -> col0 sq partial
GPSIMD is intentionally unused (its tensor ops measured ~5x slower than the
cost model on hardware); DMA loads go through the SP HWDGE ring.

The kernel is DMA-bound: compute hides fully under the HBM->SBUF stream
(~320-340 GB/s per core sustained). 6144-wide io tiles (3 MiB per DMA,
double-buffered) measured fastest; compute runs on 3072-wide sub-slices of
each landed tile so the work tiles fit SBUF. Queue splits (ACT HWDGE /
SWDGE), fp32->bf16 cast-on-DMA, and deeper buffering measured slower or
neutral.

Per-core output is a [128, 3*NT] fp32 accumulator; the host sums in float64
and combines: loss = sq_total / (B*3) + penalty_total / B.
"""
import sys

sys.path.insert(0, "/opt/trn_rl_repo")

import numpy as np

B = 8388608
C = 3
NCORES = 8
P = 128
BP = B // NCORES                   # rows per core
FLAT = BP * C                      # 3,145,728 f32 per tensor per core
PER_PART = FLAT // P               # 24,576 elements per partition
MAGIC = 12582912.0                 # 1.5 * 2**23

SIZES = [6144] * 4            # io-tile width (3 MiB DMAs)
SUB = 2                       # compute sub-slices per io tile
NT = len(SIZES) * SUB         # accumulator triplets
assert sum(SIZES) == PER_PART

_CACHE = {}


def _build_program():
    import concourse.bacc as bacc
    import concourse.tile as tile
    from concourse import mybir

    nt = NT
    nc = bacc.Bacc("TRN2", target_bir_lowering=False, debug=False)

    o_d = nc.dram_tensor("outputs", [BP, C], mybir.dt.float32, kind="ExternalInput").ap()
    t_d = nc.dram_tensor("targets", [BP, C], mybir.dt.float32, kind="ExternalInput").ap()
    acc_d = nc.dram_tensor("acc", [P, 3 * nt], mybir.dt.float32, kind="ExternalOutput").ap()

    o2 = o_d.flatten().rearrange("(p m) -> p m", p=P)
    t2 = t_d.flatten().rearrange("(p m) -> p m", p=P)

    f32 = mybir.dt.float32
    AO = mybir.AluOpType
    AF = mybir.ActivationFunctionType

    with tile.TileContext(nc) as tc:
        with (
            tc.tile_pool(name="io", bufs=2) as io_pool,
            tc.tile_pool(name="work", bufs=2) as w_pool,
            tc.tile_pool(name="fixed", bufs=1) as f_pool,
        ):
            neg_magic = f_pool.tile([P, 1], f32)
            nc.vector.memset(neg_magic[:], -MAGIC)
            acc = f_pool.tile([P, 3 * nt], f32)

            off = 0
            for k0, s in enumerate(SIZES):
                o = io_pool.tile([P, s], f32, tag="o")
                t = io_pool.tile([P, s], f32, tag="t")
                nc.sync.dma_start(o[:], o2[:, off:off + s])
                nc.sync.dma_start(t[:], t2[:, off:off + s])
                off += s

                for h in range(SUB):
                    ss = s // SUB
                    sa, s0 = ss // 3 * 2, ss // 3
                    k = k0 * SUB + h
                    orr = o[:, h * ss:(h + 1) * ss].rearrange("p (n c) -> p n c", c=3)
                    trr = t[:, h * ss:(h + 1) * ss].rearrange("p (n c) -> p n c", c=3)
                    oa, ta = orr[:, :, 1:3], trr[:, :, 1:3]
                    o0, t0 = orr[:, :, 0], trr[:, :, 0]

                    # angle squared wrap-error
                    u = w_pool.tile([P, sa], f32, tag="u")
                    nc.vector.tensor_tensor(
                        u[:].rearrange("p (n c) -> p n c", c=2), oa, ta, AO.subtract
                    )
                    r = w_pool.tile([P, sa], f32, tag="r")
                    nc.vector.tensor_scalar(r[:], u[:], MAGIC, MAGIC, AO.add, AO.subtract)
                    negd2 = w_pool.tile([P, sa], f32, tag="negd2")
                    nc.vector.tensor_tensor(negd2[:], r[:], u[:], AO.subtract)
                    nc.scalar.activation(
                        negd2[:], negd2[:], AF.Square, accum_out=acc[:, 3 * k: 3 * k + 1]
                    )

                    # penalty: |floor(o_angle)| via M + floor(o) then Abs(x - M)
                    s2 = w_pool.tile([P, sa], f32, tag="s2")
                    nc.vector.tensor_scalar(
                        s2[:].rearrange("p (n c) -> p n c", c=2),
                        oa, 0.5, MAGIC, AO.subtract, AO.add,
                    )
                    nc.scalar.activation(
                        s2[:], s2[:], AF.Abs, bias=neg_magic[:], scale=1.0,
                        accum_out=acc[:, 3 * k + 1: 3 * k + 2],
                    )

                    # col0 squared error
                    u0 = w_pool.tile([P, s0], f32, tag="u0")
                    nc.vector.tensor_tensor(u0[:], o0, t0, AO.subtract)
                    nc.scalar.activation(
                        u0[:], u0[:], AF.Square, accum_out=acc[:, 3 * k + 2: 3 * k + 3]
                    )

            nc.sync.dma_start(acc_d, acc[:])

    nc.compile()
    return nc


def _get_program():
    if "nc" not in _CACHE:
        _CACHE["nc"] = _build_program()
    return _CACHE["nc"]


def kernel(outputs: np.ndarray, targets: np.ndarray) -> np.ndarray:
    from concourse.bass_utils import run_bass_kernel_spmd

    assert outputs.shape == (B, C) and targets.shape == (B, C)
    nc = _get_program()

    o_sh = np.ascontiguousarray(np.asarray(outputs, dtype=np.float32).reshape(NCORES, BP, C))
    t_sh = np.ascontiguousarray(np.asarray(targets, dtype=np.float32).reshape(NCORES, BP, C))
    in_maps = [{"outputs": o_sh[i], "targets": t_sh[i]} for i in range(NCORES)]

    res = run_bass_kernel_spmd(nc, in_maps, core_ids=list(range(NCORES)))

    nt = NT
    sq = 0.0
    pen = 0.0
    for i in range(NCORES):
        a = res.results[i]["acc"].astype(np.float64).reshape(P, nt, 3)
        sq += a[:, :, 0].sum() + a[:, :, 2].sum()
        pen += a[:, :, 1].sum()

    result = sq / (B * C) + pen / B
    return np.float32(result)


if __name__ == "__main__":
    rng = np.random.default_rng(0)
    o = rng.standard_normal((B, C)).astype(np.float32)
    t = rng.random((B, C), dtype=np.float32)
    print(kernel(o, t))

